# revision 3
# baseline (speedup 1.0000x reference)
"""Trainium2 Bass kernel for nn_CausalAttentionPooling.

Math: scores[b,i,j] = x[b,i].q are constant along the softmax axis j, so
softmax over the causal mask yields uniform weights 1/(i+1) on j <= i.
The module is exactly a causal cumulative mean:
    out[b,i,:] = cumsum(x, axis=1)[b,i,:] / (i+1)
(q does not affect the output.)

Sharding: 8 shards = (batch b in 0..3) x (D-half dh in 0..1); each core gets
x[b, :, dh*128:(dh+1)*128] transposed to [128(D), 4096(L)].  Per core:
  - DVE tensor_tensor_scan along the free dim -> exact fp32 cumsum
  - PE 128x128 transpose tiles back to [L, D] layout
  - ScalarE activation copy PSUM->SBUF fused with per-partition 1/(i+1) scale
  - DMA out contiguous [4096, 128] slices; host reassembles
No cross-core communication.
"""

import numpy as np

B, L, D = 4, 4096, 256
NCORES = 8
P = 128           # partitions / D-shard width / L-tile height
NT = L // P       # 32 L-tiles
SCAN_BLK = 512
NBLK = L // SCAN_BLK

_cache = {}


def _split_waits_bir(bir_bytes):
    """This container's walrus build rejects instructions carrying more than
    one (or for some opcodes, two) sync waits.  Hoist multi-wait sync_info
    onto standalone same-engine EventSemaphore instructions inserted
    immediately before the instruction; program order on the engine's stream
    preserves semantics."""
    import orjson

    d = orjson.loads(bir_bytes)
    n = 0
    for fn in d["functions"]:
        for bb in fn["blocks"]:
            out = []
            for inst in bb["instructions"]:
                si = inst.get("sync_info")
                waits = (si or {}).get("on_wait") or []
                if len(waits) > 1:
                    for w in waits:
                        out.append(
                            {
                                "debug": inst.get("debug"),
                                "engine": inst["engine"],
                                "ins": [],
                                "name": f"I-waitfix-{n}",
                                "opcode": "EventSemaphore",
                                "outs": [],
                                "sync_info": {"on_wait": [w], "on_update": []},
                            }
                        )
                        n += 1
                    si["on_wait"] = []
                out.append(inst)
            bb["instructions"] = out
    return orjson.dumps(d)


def _install_bir_patch():
    if _cache.get("patched"):
        return
    import concourse.bass as bass

    orig = bass.Bass.to_json_bytes

    def patched(self):
        return _split_waits_bir(orig(self))

    bass.Bass.to_json_bytes = patched
    _cache["patched"] = True


def _build_nc():
    import concourse.bass as bass
    import concourse.tile as tile
    from concourse import mybir

    _install_bir_patch()

    f32 = mybir.dt.float32
    nc = bass.Bass()
    xT = nc.declare_dram_parameter("xT", [P, L], f32, isOutput=False)
    recip = nc.declare_dram_parameter("recip", [P, NT], f32, isOutput=False)
    ident = nc.declare_dram_parameter("ident", [P, P], f32, isOutput=False)
    out = nc.declare_dram_parameter("out", [L, P], f32, isOutput=True)

    with tile.TileContext(nc) as tc:
        with (
            tc.tile_pool(name="consts", bufs=1) as consts,
            tc.tile_pool(name="xin", bufs=NBLK) as xin,
            tc.tile_pool(name="cum", bufs=1) as cumpool,
            tc.tile_pool(name="tp", bufs=4, space="PSUM") as tp,
            tc.tile_pool(name="outp", bufs=6) as outp,
        ):
            recip_sb = consts.tile([P, NT], f32)
            nc.sync.dma_start(recip_sb[:], recip[:])
            ident_sb = consts.tile([P, P], f32)
            nc.sync.dma_start(ident_sb[:], ident[:])

            cum = cumpool.tile([P, L], f32)

            # blocked scan: cumsum along L (free dim), carried via last column
            for k in range(NBLK):
                xb = xin.tile([P, SCAN_BLK], f32)
                nc.sync.dma_start(xb[:], xT[:, k * SCAN_BLK : (k + 1) * SCAN_BLK])
                init = 0.0 if k == 0 else cum[:, k * SCAN_BLK - 1 : k * SCAN_BLK]
                nc.vector.tensor_tensor_scan(
                    cum[:, k * SCAN_BLK : (k + 1) * SCAN_BLK],
                    xb[:],
                    xb[:],
                    init,
                    op0=mybir.AluOpType.add,
                    op1=mybir.AluOpType.bypass,
                )

            # per 128-col tile: transpose to [L-rows, D] then scale by 1/(i+1)
            for t in range(NT):
                ps = tp.tile([P, P], f32)
                nc.tensor.transpose(ps[:], cum[:, t * P : (t + 1) * P], ident_sb[:])
                ob = outp.tile([P, P], f32)
                nc.scalar.activation(
                    ob[:],
                    ps[:],
                    mybir.ActivationFunctionType.Copy,
                    scale=recip_sb[:, t : t + 1],
                )
                nc.sync.dma_start(out[t * P : (t + 1) * P, :], ob[:])
    return nc


def _get_nc():
    if "nc" not in _cache:
        _cache["nc"] = _build_nc()
    return _cache["nc"]


def kernel(x, q):
    from concourse.bass_utils import run_bass_kernel_spmd

    x = np.asarray(x)
    assert x.shape == (B, L, D) and x.dtype == np.float32

    nc = _get_nc()

    idx = np.arange(1, L + 1, dtype=np.float64)
    recip_pt = np.ascontiguousarray(
        (1.0 / idx).astype(np.float32).reshape(NT, P).T
    )  # [P, NT]: recip_pt[p, t] = 1/(t*128+p+1)
    ident = np.eye(P, dtype=np.float32)

    in_maps = []
    shard_of_core = []
    for c in range(NCORES):
        b, dh = c // 2, c % 2
        shard_of_core.append((b, dh))
        xT = np.ascontiguousarray(x[b, :, dh * P : (dh + 1) * P].T)
        in_maps.append({"xT": xT, "recip": recip_pt, "ident": ident})

    results = run_bass_kernel_spmd(nc, in_maps, list(range(NCORES))).results

    out = np.empty((B, L, D), dtype=np.float32)
    for c, (b, dh) in enumerate(shard_of_core):
        out[b, :, dh * P : (dh + 1) * P] = results[c]["out"]
    return out


# revision 4
# speedup vs baseline: 1.1670x; 1.1670x over previous
"""Trainium2 Bass kernel for nn_CausalAttentionPooling.

Math: scores[b,i,j] = x[b,i].q are constant along the softmax axis j, so
softmax over the causal mask yields uniform weights 1/(i+1) on j <= i.
The module is exactly a causal cumulative mean:
    out[b,i,:] = cumsum(x, axis=1)[b,i,:] / (i+1)
(q does not affect the output.)

Sharding: 8 shards = (batch b in 0..3) x (D-half dh in 0..1); each core gets
x[b, :, dh*128:(dh+1)*128] transposed to [128(D), 4096(L)].  Per core:
  - DVE tensor_tensor_scan along the free dim -> exact fp32 cumsum
  - DVE tensor_tensor multiply by a replicated 1/(i+1) row (host-shipped)
  - DMA out [128(D), 4096(L)]; host transposes slices back
No cross-core communication; DMA count kept minimal (issue cost ~600ns each).
"""

import numpy as np

B, L, D = 4, 4096, 256
NCORES = 8
P = 128            # partitions / D-shard width
BLK = 1024         # scan/mult block along L
NBLK = L // BLK

_cache = {}


def _split_waits_bir(bir_bytes):
    """This container's walrus build rejects instructions carrying more than
    one (or for some opcodes, two) sync waits.  Hoist multi-wait sync_info
    onto standalone same-engine EventSemaphore instructions inserted
    immediately before the instruction; program order on the engine's stream
    preserves semantics."""
    import orjson

    d = orjson.loads(bir_bytes)
    n = 0
    for fn in d["functions"]:
        for bb in fn["blocks"]:
            out = []
            for inst in bb["instructions"]:
                si = inst.get("sync_info")
                waits = (si or {}).get("on_wait") or []
                if len(waits) > 1:
                    for w in waits:
                        out.append(
                            {
                                "debug": inst.get("debug"),
                                "engine": inst["engine"],
                                "ins": [],
                                "name": f"I-waitfix-{n}",
                                "opcode": "EventSemaphore",
                                "outs": [],
                                "sync_info": {"on_wait": [w], "on_update": []},
                            }
                        )
                        n += 1
                    si["on_wait"] = []
                out.append(inst)
            bb["instructions"] = out
    return orjson.dumps(d)


def _install_bir_patch():
    if _cache.get("patched"):
        return
    import concourse.bass as bass

    orig = bass.Bass.to_json_bytes

    def patched(self):
        return _split_waits_bir(orig(self))

    bass.Bass.to_json_bytes = patched
    _cache["patched"] = True


def _build_nc():
    import concourse.bass as bass
    import concourse.tile as tile
    from concourse import mybir

    _install_bir_patch()

    f32 = mybir.dt.float32
    add = mybir.AluOpType.add
    byp = mybir.AluOpType.bypass
    mult = mybir.AluOpType.mult

    nc = bass.Bass()
    xT = nc.declare_dram_parameter("xT", [P, L], f32, isOutput=False)
    rr = nc.declare_dram_parameter("rr", [P, L], f32, isOutput=False)
    out = nc.declare_dram_parameter("out", [P, L], f32, isOutput=True)

    with tile.TileContext(nc) as tc:
        with tc.tile_pool(name="sb", bufs=1) as sb:
            xt = sb.tile([P, L], f32, tag="xt")
            rt = sb.tile([P, L], f32, tag="rt")
            cum = sb.tile([P, L], f32, tag="cum")
            ot = sb.tile([P, L], f32, tag="ot")
            nc.sync.dma_start(rt[:], rr[:])
            for k in range(NBLK):
                s = slice(k * BLK, (k + 1) * BLK)
                nc.sync.dma_start(xt[:, s], xT[:, s])
                init = 0.0 if k == 0 else cum[:, k * BLK - 1 : k * BLK]
                nc.vector.tensor_tensor_scan(
                    cum[:, s], xt[:, s], xt[:, s], init, op0=add, op1=byp
                )
                nc.vector.tensor_tensor(ot[:, s], cum[:, s], rt[:, s], op=mult)
                nc.sync.dma_start(out[:, s], ot[:, s])
    return nc


def _get_nc():
    if "nc" not in _cache:
        _cache["nc"] = _build_nc()
    return _cache["nc"]


def _make_in_maps(x):
    idx = np.arange(1, L + 1, dtype=np.float64)
    recip_rep = np.ascontiguousarray(
        np.broadcast_to((1.0 / idx).astype(np.float32), (P, L))
    )
    in_maps = []
    shards = []
    for c in range(NCORES):
        b, dh = c // 2, c % 2
        shards.append((b, dh))
        xT = np.ascontiguousarray(x[b, :, dh * P : (dh + 1) * P].T)
        in_maps.append({"xT": xT, "rr": recip_rep})
    return in_maps, shards


def kernel(x, q):
    from concourse.bass_utils import run_bass_kernel_spmd

    x = np.asarray(x)
    assert x.shape == (B, L, D) and x.dtype == np.float32

    nc = _get_nc()
    in_maps, shards = _make_in_maps(x)
    results = run_bass_kernel_spmd(nc, in_maps, list(range(NCORES))).results

    out = np.empty((B, L, D), dtype=np.float32)
    for c, (b, dh) in enumerate(shards):
        out[b, :, dh * P : (dh + 1) * P] = results[c]["out"].T
    return out
